# revision 26
# baseline (speedup 1.0000x reference)
"""MoE SwiGLU FFN (E=8, top-2) Trainium2 kernel — expert parallelism over 8 cores.

Contract: kernel(**inputs) takes the FULL inputs from setup_inputs() and
returns (out [4,2048,1024] f32, aux_loss f32), matching reference().

Strategy
--------
Host (numpy, f64): router (logits/softmax/top-2/combine weights + aux_loss).
Selections match jax's f32 top_k (margins >> f32 noise; verified on seed-0 data).

Device (8 NeuronCores, SPMD, one program): core e owns expert e.
- Inputs per core: full x (gather source), the expert's token index list
  (padded to TCAP), combine weights, and the expert's w1/w3/w2 transposed on
  host to matmul-friendly layouts (w1T/w3T = [C,H], w2T = [H,C]).
- Tokens are gathered with indirect DMA ([128,C] rows), transposed to [C,t]
  with PE transposes (fp32 has no DMA transpose), and run through the FFN in
  fp32r (TF32-like: 1 PE cycle/row at free dim >= 256; measured rel err
  ~1.5e-4 per 1024-contraction matmul).
- H=2816 is split into two halves per SBUF capacity; each half-pass scatters
  its partial y (scaled by combine weight) into a zeroed per-core [N,C] DRAM
  buffer with accumulating indirect DMA.
- ReduceScatter(add) over the 8 cores yields each core's [N/8, C] shard of the
  final output; host concatenates.

fp32/fp32r matmuls self-load weights and have a single sync-wait slot; the
bacc layer (generate_event_semaphores) splits excess waits automatically.
"""

import os

import numpy as np

import concourse.bass as bass
import concourse.mybir as mybir
import concourse.tile as tile
from concourse import bacc
from concourse.bass_utils import run_bass_kernel_spmd
from concourse.masks import make_identity

E = 8
TOP_K = 2
C = 1024
H = 2816
HHALF = H // 2          # 1408
HC_HALF = HHALF // 128  # 11 h-chunks of 128 per half
CC = C // 128           # 8 c-chunks of 128
TB = 256                # tokens per block (fp32r needs free dim >= 256)
N_CORES = 8
F32 = mybir.dt.float32
F32R = mybir.dt.float32r
I32 = mybir.dt.int32
I16 = mybir.dt.int16


def _routing(x, gate_w):
    """Host router in f64. Returns combine [N,E], aux_loss (f32 scalar)."""
    xf = x.reshape(-1, C).astype(np.float64)
    logits = xf @ gate_w.astype(np.float64).T
    logits -= logits.max(axis=-1, keepdims=True)
    p = np.exp(logits)
    p /= p.sum(axis=-1, keepdims=True)
    order = np.argsort(-p, axis=-1, kind="stable")  # ties -> lower index first
    top2 = order[:, :TOP_K]                          # [N, K]
    n = p.shape[0]
    top_w = p[np.arange(n)[:, None], top2]
    top_w = top_w / top_w.sum(axis=-1, keepdims=True)
    combine = np.zeros((n, E), dtype=np.float64)
    np.put_along_axis(combine, top2, top_w, axis=1)
    mask = (combine > 0).astype(np.float64)
    aux = E * float(np.sum(p.mean(axis=0) * mask.mean(axis=0)))
    return combine, np.float32(aux)


def _build(ntok, tcap, b_end, b_start, collective=True, dbg=()):
    NCH = len(b_end)
    nc = bacc.Bacc("TRN2", target_bir_lowering=False, debug=False,
                   num_devices=N_CORES, num_swdge_queues=2)
    x_in = nc.declare_dram_parameter("x", [ntok, C], F32, isOutput=False)
    idxc_in = [nc.declare_dram_parameter(f"idx16c{k}", [128, tcap // 16], I16,
                                         isOutput=False) for k in range(NCH)]
    idxg_in = nc.declare_dram_parameter("idx16g", [128, tcap // 16], I16, isOutput=False)
    cw_in = nc.declare_dram_parameter("cw", [tcap, 1], F32, isOutput=False)
    w1t_in = nc.declare_dram_parameter("w1t", [C, H], F32R, isOutput=False)
    w3t_in = nc.declare_dram_parameter("w3t", [C, H], F32R, isOutput=False)
    w2t_in = nc.declare_dram_parameter("w2t", [H, C], F32R, isOutput=False)
    y_out = nc.declare_dram_parameter("y", [ntok // N_CORES, C], F32,
                                      isOutput=True)

    qsz = ntok // NCH
    y_c = [nc.dram_tensor(f"y_c{k}", [qsz + 128, C], F32) for k in range(NCH)]
    xt_dram = nc.dram_tensor("xt_dram", [C, tcap], F32R)
    rs_c = [nc.dram_tensor(f"rs_c{k}", [qsz // N_CORES, C], F32)
            for k in range(NCH)]

    xt_dram_v = xt_dram.ap().rearrange("(co ci) t -> ci co t", ci=128)
    y_c_v = [t.ap()[:qsz, :].rearrange("(a p) c -> p a c", p=128) for t in y_c]
    n_blocks = tcap // TB

    with tile.TileContext(nc) as tc:
        with (
            tc.tile_pool(name="wpool", bufs=1) as wpool,
            tc.tile_pool(name="xpool", bufs=2) as xpool,
            tc.tile_pool(name="gpool", bufs=2) as gpool,
            tc.tile_pool(name="spool", bufs=2) as spool,
            tc.tile_pool(name="zpool", bufs=1) as zpool,
            tc.tile_pool(name="gpool1", bufs=1) as gpool1,
            tc.tile_pool(name="psA", bufs=2, space="PSUM") as psA,
            tc.tile_pool(name="psB", bufs=2, space="PSUM") as psB,
            tc.tile_pool(name="psY", bufs=1, space="PSUM") as psY,
            tc.tile_pool(name="psT", bufs=2, space="PSUM") as psT,
        ):
            # --- zero y_part (scatters accumulate into it) ---
            zero_t = zpool.tile([128, C // 2], F32)
            nc.vector.memset(zero_t[:], 0.0)

            def emit_zero_chunk(k, gate=None):
                if "nozero" in dbg:
                    return
                for a in range(qsz // 128):
                    for hf in range(2):
                        inst = nc.sync.dma_start(
                            y_c_v[k][:, a, hf * 512:(hf + 1) * 512], zero_t[:])
                        if gate is not None:
                            tile.add_dep_helper(
                                inst.ins, gate.ins, sync=True,
                                reason="defer chunk zero past early gathers")

            ident = zpool.tile([128, 128], F32)
            make_identity(nc, ident[:])
            idxg_t = zpool.tile([128, tcap // 16], I16)
            nc.sync.dma_start(idxg_t[:], idxg_in.ap()[:])
            idxc_t = []
            for k in range(NCH):
                t = zpool.tile([128, tcap // 16], I16, name=f"idxc{k}")
                nc.sync.dma_start(t[:], idxc_in[k].ap()[:])
                idxc_t.append(t)

            for p in range(2):  # h-half passes
                hsl = slice(p * HHALF, (p + 1) * HHALF)
                w1h = wpool.tile([128, CC, HHALF], F32R, tag="w1h")
                w3h = wpool.tile([128, CC, HHALF], F32R, tag="w3h")
                w2h = wpool.tile([128, HC_HALF, C], F32R, tag="w2h")
                w1t_v = w1t_in.ap()[:, hsl].rearrange("(co ci) h -> ci co h", ci=128)
                w3t_v = w3t_in.ap()[:, hsl].rearrange("(co ci) h -> ci co h", ci=128)
                for hc in range(HC_HALF):
                    h128 = slice(hc * 128, (hc + 1) * 128)
                    nc.sync.dma_start(w1h[:, :, h128], w1t_v[:, :, h128])
                    nc.sync.dma_start(w3h[:, :, h128], w3t_v[:, :, h128])
                    row0 = (p * HC_HALF + hc) * 128
                    nc.sync.dma_start(
                        w2h[:, hc, :],
                        w2t_in.ap()[row0:row0 + 128, :])
                if p == 0:
                    gather_inst = [None]

                    def gather_tile(tbx):
                        t = gpool1.tile([128, TB // 128, C], F32, tag="xg",
                                        name=f"xg{tbx}")
                        gather_inst[0] = nc.gpsimd.dma_gather(
                            out_ap=t[:], in_ap=x_in.ap()[:],
                            idxs_ap=idxg_t[:, tbx * 16:(tbx + 1) * 16],
                            num_idxs=TB, num_idxs_reg=TB, elem_size=C,
                            queue_num=0)
                        return t

                    xg_next = gather_tile(0)
                    emit_zero_chunk(0)
                    emit_zero_chunk(1)

                for tb in range(n_blocks):
                    tsl = slice(tb * TB, (tb + 1) * TB)
                    xt_t = xpool.tile([128, CC, TB], F32R, tag="xt")
                    cw_ts = []
                    for sub in range(TB // 128):
                        g0 = tb * TB + sub * 128
                        cw_t = spool.tile([128, 1], F32, tag="cw")
                        nc.scalar.dma_start(cw_t[:], cw_in.ap()[g0:g0 + 128, :])
                        cw_ts.append(cw_t)

                    if p == 0:
                        # transpose the block gathered during the previous
                        # iteration; prefetch the next block's gather ahead
                        # of this block's scatter (avoids Pool head-of-line)
                        xg = xg_next
                        for sub in range(TB // 128):
                            for c in range(CC):
                                pst = psT.tile([128, 128], F32, tag="pst")
                                nc.tensor.transpose(
                                    pst[:], xg[:, sub, c * 128:(c + 1) * 128],
                                    ident[:])
                                nc.vector.tensor_copy(
                                    xt_t[:, c, sub * 128:(sub + 1) * 128],
                                    pst[:])
                        nc.scalar.dma_start(xt_dram_v[:, :, tsl], xt_t[:])
                        for k in range(2, NCH):
                            if tb == max(0, b_start[k] - 1):
                                emit_zero_chunk(k, gate=gather_inst[0])
                        if tb + 1 < n_blocks:
                            xg_next = gather_tile(tb + 1)
                    else:
                        nc.scalar.dma_start(xt_t[:], xt_dram_v[:, :, tsl])

                    # mm1/mm3 + swiglu -> G^T [h, t] (fp32r)
                    g_t = gpool.tile([128, HC_HALF, TB], F32R, tag="g")
                    for hc in range(HC_HALF):
                        h128 = slice(hc * 128, (hc + 1) * 128)
                        pa = psA.tile([128, TB], F32, tag="pa")
                        pb = psB.tile([128, TB], F32, tag="pb")
                        for c in range(CC):
                            nc.tensor.matmul(pa[:], lhsT=w1h[:, c, h128],
                                             rhs=xt_t[:, c, :],
                                             start=(c == 0), stop=(c == CC - 1))
                        for c in range(CC):
                            nc.tensor.matmul(pb[:], lhsT=w3h[:, c, h128],
                                             rhs=xt_t[:, c, :],
                                             start=(c == 0), stop=(c == CC - 1))
                        act = spool.tile([128, TB], F32, tag="act")
                        nc.scalar.activation(
                            act[:], pa[:], mybir.ActivationFunctionType.Silu)
                        nc.vector.tensor_mul(
                            out=g_t[:, hc, :], in0=act[:], in1=pb[:])

                    # mm2: y[t, c] = G^T.T @ w2T, scale by cw, scatter-add
                    y_t = spool.tile([128, TB // 128, C], F32, tag="y")
                    for sub in range(TB // 128):
                        s128 = slice(sub * 128, (sub + 1) * 128)
                        py = [psY.tile([128, 512], F32, tag=f"py{cc}", name=f"py{cc}_{p}_{tb}_{sub}")
                              for cc in range(2)]
                        for hc in range(HC_HALF):
                            for cc in range(2):
                                nc.tensor.matmul(
                                    py[cc][:],
                                    lhsT=g_t[:, hc, s128],
                                    rhs=w2h[:, hc, cc * 512:(cc + 1) * 512],
                                    start=(hc == 0), stop=(hc == HC_HALF - 1))
                        for cc in range(2):
                            nc.scalar.activation(
                                y_t[:, sub, cc * 512:(cc + 1) * 512], py[cc][:],
                                mybir.ActivationFunctionType.Copy,
                                scale=cw_ts[sub][:, :1])
                    for k in range(NCH):
                        if b_start[k] <= tb < b_end[k]:
                            nc.gpsimd.dma_scatter_add(
                                out_ap=y_c[k].ap()[:], in_ap=y_t[:],
                                idxs_ap=idxc_t[k][:, tb * 16:(tb + 1) * 16],
                                num_idxs=TB, num_idxs_reg=TB, elem_size=C,
                                queue_num=1)
                        if (collective and p == 1 and k < NCH - 1
                                and tb == b_end[k] - 1):
                            nc.gpsimd.collective_compute(
                                "ReduceScatter",
                                mybir.AluOpType.add,
                                replica_groups=[list(range(N_CORES))],
                                ins=[y_c[k].ap()[:qsz, :]],
                                outs=[rs_c[k].ap()[:]],
                            )
                            sh = qsz // N_CORES
                            nc.sync.dma_start(
                                y_out.ap()[k * sh:(k + 1) * sh, :],
                                rs_c[k].ap()[:])

            if collective:
                k = NCH - 1
                nc.gpsimd.collective_compute(
                    "ReduceScatter",
                    mybir.AluOpType.add,
                    replica_groups=[list(range(N_CORES))],
                    ins=[y_c[k].ap()[:qsz, :]],
                    outs=[rs_c[k].ap()[:]],
                )
                sh = qsz // N_CORES
                nc.sync.dma_start(
                    y_out.ap()[k * sh:(k + 1) * sh, :], rs_c[k].ap()[:])
            else:
                nc.sync.dma_start(y_out.ap()[:],
                                  y_c[0].ap()[:ntok // N_CORES, :])

    nc.compile()
    return nc


def kernel(x, gate_w, w1, w2, w3):
    x = np.asarray(x, dtype=np.float32)
    gate_w = np.asarray(gate_w, dtype=np.float32)
    w1 = np.asarray(w1, dtype=np.float32)
    w2 = np.asarray(w2, dtype=np.float32)
    w3 = np.asarray(w3, dtype=np.float32)
    B, T, _ = x.shape
    ntok = B * T
    xf = np.ascontiguousarray(x.reshape(ntok, C))

    combine, aux = _routing(x, gate_w)

    idx_list, cw_list = [], []
    for e in range(E):
        (tok,) = np.nonzero(combine[:, e])
        idx_list.append(tok)
        cw_list.append(combine[tok, e].astype(np.float32))
    tcap = max(TB, int(-(-max(len(t) for t in idx_list) // TB) * TB))

    NCH = 4
    qsz = ntok // NCH
    garbage = qsz  # redirect row in each chunk tensor
    in_maps = []
    # pos[e][k]: first position whose token id >= qsz*k (per core)
    pos = np.zeros((E, NCH + 1), dtype=np.int64)
    for e in range(E):
        idx = np.zeros(tcap, dtype=np.int64)
        cw = np.zeros((tcap, 1), dtype=np.float32)
        ne = len(idx_list[e])
        idx[:ne] = idx_list[e]
        cw[:ne, 0] = cw_list[e]
        valid = np.zeros(tcap, dtype=bool)
        valid[:ne] = True

        def wrap(a):
            return np.ascontiguousarray(
                np.tile(a.reshape(tcap // 16, 16).T, (8, 1)))

        m = {
            "x": xf,
            "idx16g": wrap(idx.astype(np.int16)),  # gather: pads read row 0
            "cw": cw,
            "w1t": np.ascontiguousarray(w1[e].T),
            "w3t": np.ascontiguousarray(w3[e].T),
            "w2t": np.ascontiguousarray(w2[e].T),
        }
        for k in range(NCH):
            in_k = valid & (idx >= qsz * k) & (idx < qsz * (k + 1))
            m[f"idx16c{k}"] = wrap(
                np.where(in_k, idx - qsz * k, garbage).astype(np.int16))
            pos[e, k] = int(np.argmax(valid & (idx >= qsz * k))) \
                if (valid & (idx >= qsz * k)).any() else ne
        pos[e, NCH] = ne
        in_maps.append(m)

    b_end = tuple(max(1, int(max(-(-pos[e, k + 1] // TB) for e in range(E))))
                  for k in range(NCH))
    b_start = tuple(int(min(pos[e, k] // TB for e in range(E)))
                    for k in range(NCH))
    globals()["LAST_TCAP"] = tcap
    globals()["LAST_BOUNDS"] = (b_end, b_start)
    nc = _build(ntok, tcap, b_end, b_start)
    res = run_bass_kernel_spmd(nc, in_maps, list(range(N_CORES)),
                               trace=bool(os.environ.get("KERNEL_TRACE")))
    globals()["LAST_EXEC_NS"] = res.exec_time_ns
    sh = qsz // N_CORES
    out = np.empty((ntok, C), dtype=np.float32)
    for i in range(N_CORES):
        y = res.results[i]["y"]
        for k in range(NCH):
            out[qsz * k + sh * i:qsz * k + sh * (i + 1)] = \
                y[k * sh:(k + 1) * sh]
    return out.reshape(B, T, C), aux


if __name__ == "__main__":
    rng = np.random.default_rng(0)
    inputs = {
        "x": rng.standard_normal((4, 2048, C), dtype=np.float32),
        "gate_w": (0.02 * rng.standard_normal((E, C))).astype(np.float32),
        "w1": (0.02 * rng.standard_normal((E, H, C))).astype(np.float32),
        "w2": (0.02 * rng.standard_normal((E, C, H))).astype(np.float32),
        "w3": (0.02 * rng.standard_normal((E, H, C))).astype(np.float32),
    }
    out, aux = kernel(**inputs)
    print(out.shape, aux)
